# revision 64
# baseline (speedup 1.0000x reference)
"""Windowed 3D attention (nn_Attention3d) Trainium2 kernel, 8-core SPMD.

Sharding: the 8x8 grid of 16x16 spatial windows is split by row across the
8 NeuronCores. Each core processes a (C=256, F=2, 16, W=128) slab of x:
8 windows x 8 heads of independent 512-token attention, plus the QKV and
output projections for its slab. Host-side numpy does the window
permutation and weight transposes; cores see dense (C, tokens) panels.

v2 (this session): all matmul operands bf16 (x/weights converted host-side;
1.0 vs 1.5 PE cyc/row, x DMA halved; fp32 PSUM accumulate keeps rel err at
5.3e-3), and the PSUM-evacuation copies load-balanced DVE<->ACT: the V-proj
evacuations plus Q/K-proj evacuations {0,4,2,7} run as scalar-engine copies
(the DVE was the busiest engine at 94% in the scheduling-cost-model trace;
ACT had slack after its exp stream). Engine busy% after rebalance is
PE/DVE/ACT ~ 88/84/82. The y bias-add is priority-deferred so a blocked
DVE queue head never idles the engine at the window boundary. Measured by
interleaved repeat-differencing: 263.8k -> ~197-216k ns/pass (machine-load
dependent; shared-box drift is +-7%, so only same-process interleaved A/B
comparisons (cmp.py) are trustworthy). Dead ends (all HW-verified, do not
retry): custom-DVE reciprocal reading PSUM partition 64 (offset bug ->
garbage), any DVE op on partitions starting at 1 (BIR verifier rejects;
DVE write bases are {0,32,64,96} while gpsimd broadcast reads partitions
0-15 only, so partition 0 is the ONLY denominator staging point), bf16
PSUM (TRN3-only), gpsimd anything-PSUM (no port), UBIG [128,2048] exps
(+16k/pass: sim_ps serialization beats the ACT-overhead saving), CHAIN_HP=0
(+10k), Y_LAG=2 (+12k), PSUM pool rebalances off 2/2/4 (all much worse),
fp8 anywhere (error budget). Same-process HW ablations: nonorm -20k/pass,
halfexp -10k/pass -- the normalize chain and ACT exp stream are the
remaining critical-path residents, but every restructure of them runs into
the partition-alignment walls above. Round-3 sweeps (all same-process HW,
all neutral or worse): QK_ACT_CHUNKS {0,4,2,6}/{0,4,3,7}/{1,5,2,7}/
{0,5,2,7}, START_EARLY (-0.9k, sub-noise), PROJ/O PSUM bufs off 2/2.
WARM_MMS trimmed 26->10 (steady-state neutral; ~7us less repeat=1
preamble, still covers the ~3.4us HAM SHORT window). The normalize chain
is at its op-count floor: s-row copy (641ns, only PSUM->part-0 path),
recip (594ns, fp32 [1,512] custom-DVE), bcast (GP), mult (658ns,
doubles as the po evacuation). Round 4: repeat<=body_passes now emits
straight-line (no For_i machinery on the harness's repeat=1 dispatch;
steady metric unchanged). V_ACT_TS=0,1 (v-copy ACT/DVE split) predicted
-4k by the sim but measured +26k on HW in every interleaved iteration --
the sharpest sim-HW inversion seen; v-copies stay all-ACT. Trust only
same-process interleaved A/B (cmp.py) for accept/reject decisions.
Round 5: SPLIT_QK0 / QK0_HP / PROJ0_HP boundary tweaks all sim-neutral
or worse (flags kept, default off). NOTE: one correctness run returned
rel err 0.85 with a byte-identical binary and the immediate rerun passed
at 5.26e-3 -- the shared device occasionally executes a garbage run;
always rerun before diagnosing a "regression".

Kernel structure (per window; bf16 matmuls everywhere, fp32 PSUM
accumulation):
- Q/K projected inner-major (lhsT = wqk^T chunks), V projected token-major
  (lhsT = x panel), so attention needs no transposes anywhere:
  simT[j,i] = K^T-slice^T @ Q^T directly, with the two heads of an inner
  chunk row-packed into complementary 64-partition PE row groups writing
  one 2-bank PSUM tile.
- U = exp(simT/8) on ACT (one [128,1024] activation per head pair).
- O^T = (V|1)^T @ U accumulated over j; the appended ones column yields the
  softmax denominators s as PSUM row 64 for free.
- Normalization (whole chain at high scheduler priority): a DVE copy
  stages the s row to SBUF base partition 0 (the approx-recip custom op
  misreads nonzero partition offsets), then reciprocal_approx_fast
  (~51 ULP, ~5x faster than the full DVE reciprocal, which measures
  ~3.5us/instruction on HW and dominated earlier builds) -> GPSIMD
  partition_broadcast -> a DVE mult that reads po straight from PSUM
  (mixed PSUM+SBUF operands are legal) and frees the po slot. Deep
  rpool buffering (6) lets chains lag without blocking the O matmuls.
- Output projection consumes O^T chunks as lhsT; two i-tiles pack into one
  PSUM bank; y tiles allocate from proj_ps (idle then) so they never block
  po allocations; bias added on DVE; strided DMA writes (token, channel).
- The output projection is software-pipelined one window behind; x panels
  prefetch two windows ahead on the sync DMA queue.
- The whole 8-window pass is emitted once inside a hardware For_i loop over
  `repeat` (optionally `body_passes` copies per iteration), so program size
  -- and per-dispatch client overhead -- is constant in `repeat`, letting
  repeat-differencing measure true steady-state device time.

Measured (8 NeuronCores, axon, repeat-differencing at R=6144, B=4):
~243 us steady-state per pass, rel err ~2.8e-3 (bf16 attention) /
~3.3e-4 (BF16_ATT=0) vs the fp32 reference.
"""

import os
import sys

sys.path.insert(0, "/opt/trn_rl_repo")
os.environ.setdefault("MYCRO_LOCAL_CACHE", "1")

import numpy as np
from contextlib import ExitStack

import concourse.bass as bass  # noqa: F401  (AP types)
import concourse.bacc as bacc
from concourse import mybir, tile
from concourse.alu_op_type import AluOpType
from concourse.bass_utils import run_bass_kernel_spmd

FP = mybir.dt.float32
HEADS = 8
DH = 64
WS = 16
C = 256
INNER = 512  # HEADS * DH
F = 2
H = 128
W = 128
NW = 8  # windows per core (one window-grid row)
T = F * WS * WS  # 512 tokens per window
NPIX = NW * T  # 4096 pixels per core slab
XK = 2  # C=256 -> 2 partition chunks of 128
N_CORES = 8
SCALE = DH ** -0.5
MR = mybir.dt.float32r  # rounded fp32: 1.5 vs 2.0 PE cyc/row; producers must emit it
BF16_ATT = os.environ.get("BF16_ATT", "1") == "1"  # bf16 sim/O matmul operands
BA = mybir.dt.bfloat16 if BF16_ATT else MR
# bf16 projections: x / wqk / wv / wo arrive in DRAM as bf16, all matmuls run
# bf16 (1.0 vs 1.5 PE cyc/row) and the x DMA halves. fp32 PSUM accumulate.
PROJ_BF16 = os.environ.get("PROJ_BF16", "1") == "1"
PJ = mybir.dt.bfloat16 if PROJ_BF16 else MR
# DVE<->ACT balance: the DVE is the busiest engine (PSUM evacuations); ACT
# has slack after its exp stream. Move the V-proj evac copies and the first
# N_QK_ACT of the 8 Q/K evac copies per window to the scalar engine.
V_ACT = os.environ.get("V_ACT", "1") == "1"
# which of the 4 V-evac copies go to ACT when V_ACT (csv of t in 0..3)
V_ACT_TS = frozenset(
    int(x) for x in os.environ.get("V_ACT_TS", "0,1,2,3").split(",") if x != ""
)
HP_BCAST = os.environ.get("HP_BCAST", "1") == "1"
RPOOL_BUFS = int(os.environ.get("RPOOL_BUFS", "6"))
# explicit Q/K->ACT chunk list (csv of chunk ids 0-7; 0-3 Q, 4-7 K).
# {0,4,2,7} measured best on HW (interleaved A/B) and in the cost-model sim.
_qk_env = os.environ.get("QK_ACT_CHUNKS", "0,4,2,7")
QK_ACT_CHUNKS = frozenset(int(x) for x in _qk_env.split(",") if x != "")
# y bias-add scheduling: "normal" | "hp" (asap) | "defer" (after other DVE work).
# Under Y_MS=(0,1) defer won by ~10k. Under Y_MS=(0,2): one A/B showed hp
# -7.5k, the replication run showed a tie (<1k) -- treat as tied, hp kept
# as the weakly-favored default. Knobs interact; retune together.
Y_TT_PRIO = os.environ.get("Y_TT_PRIO", "hp")
# split the window-boundary pair-0 Q/K evac copies into DVE+ACT halves so
# the next window's first sims see their qk tiles ~2x sooner
SPLIT_QK0 = os.environ.get("SPLIT_QK0", "0") == "1"
# high_priority on the pair-0 boundary chain (proj matmuls / evac copies)
QK0_HP = os.environ.get("QK0_HP", "0") == "1"
PROJ0_HP = os.environ.get("PROJ0_HP", "0") == "1"
# reciprocal_approx_fast reading the s row straight from PSUM partition 64
# (skips the DVE s1 staging copy). Needs HW validation: the custom op was
# previously reported to misread nonzero partition offsets.
RECIP_PSUM = os.environ.get("RECIP_PSUM", "0") == "1"
# one [128, 4T] psim tile per half-pair: exp runs as [128, 2048] (8 ACT
# instructions/window instead of 16 -> less ACT per-instruction overhead)
UBIG = os.environ.get("UBIG", "0") == "1"
# ones column FIRST in each v 65-slot: the softmax denominator row lands at
# PSUM partition 0 (custom-op-readable without the offset bug); o rows land
# at partitions 1:65 and the normalize mult shifts partitions down by one.
V_ONES_FIRST = os.environ.get("V_ONES_FIRST", "0") == "1"
SROW = 0 if V_ONES_FIRST else 64  # po row holding the denominators
ORO = 1 if V_ONES_FIRST else 0  # first po row holding o values
# emit next pair's sims in two halves around this pair's first O group
SIMS_SPLIT = os.environ.get("SIMS_SPLIT", "0") == "1"
# emit next window's pair-0 projection at m=2 (sims stay at m=3)
START_EARLY = os.environ.get("START_EARLY", "0") == "1"
# which m iterations carry the prev window's two y-projection groups.
# (0,2) spreads them a full pair apart: measured -6..-8k ns/pass vs (0,1)
# in same-process interleaved A/B (replicated twice); NOT additive with
# deeper x/qk/u pools (each helps ~-6k alone, combined is worse than base).
Y_MS = tuple(int(x) for x in os.environ.get("Y_MS", "0,2").split(","))
# batch the two denominators of a head pair into one [2, T] tile and run a
# single [2, T] reciprocal per pair (DVE cycles scale with free size only).
# Needs: DVE copy to SBUF partition 1, gpsimd broadcast reading partition 1.
S_PAIR = os.environ.get("S_PAIR", "0") == "1"
# high_priority on the normalize chain (s1 copy / recip / final mult). When
# off, the chain schedules at natural priority, giving the strict per-engine
# queues more slack before dependency-blocked chain ops reach the head.
CHAIN_HP = os.environ.get("CHAIN_HP", "1") == "1"
# how many windows the output projection lags behind (1 = classic; 2 gives
# the o_sb normalize chain a full extra window of slack before y reads it)
Y_LAG = int(os.environ.get("Y_LAG", "1"))
# HAM warmup matmul count: enough to hold the PE busy past the ~3.4us SHORT
# window during the initial DMA wait; extra ones only stretch the repeat=1
# dispatch span (they are amortized away in the steady-state For_i metric)
WARM_MMS = int(os.environ.get("WARM_MMS", "10"))

from contextlib import nullcontext as _nullctx  # noqa: E402
# timing-only ablation knobs (comma list): nox,nostore,nonorm,halfexp,noy
ABL = set(os.environ.get("ABL", "").split(",")) - {""}


def _r(ap):
    return ap

_CACHE = {}


def _build(repeat=1, use_loop=True, body_passes=1):
    nc = bacc.Bacc("TRN2", target_bir_lowering=False, debug=False)

    xw = nc.dram_tensor("xw", [C, NPIX], PJ, kind="ExternalInput").ap()
    wqkT = nc.dram_tensor("wqkT", [C, 2 * INNER], PJ, kind="ExternalInput").ap()
    wvT = nc.dram_tensor("wvT", [C, INNER], PJ, kind="ExternalInput").ap()
    woT = nc.dram_tensor("woT", [INNER, C], PJ, kind="ExternalInput").ap()
    bo = nc.dram_tensor("bo", [1, C], FP, kind="ExternalInput").ap()
    out = nc.dram_tensor("out", [NW, T, C], FP, kind="ExternalOutput").ap()

    Exp = mybir.ActivationFunctionType.Exp

    with tile.TileContext(nc) as tc, ExitStack() as ctx:
        def pool(name, bufs, space="SBUF"):
            return ctx.enter_context(tc.tile_pool(name=name, bufs=bufs, space=space))

        consts = pool("consts", 1)
        xpool = pool("x", int(os.environ.get("XPOOL", "3")))
        qkpool = pool("qk", int(os.environ.get("QKPOOL", "3")))
        vpool = pool("v", 3)
        upool = pool("u", int(os.environ.get("UPOOL", "3")))
        rpool = pool("r", RPOOL_BUFS)  # s1/r1/rb: deep bufs let normalize chains lag
        opool = pool("o", 1 + Y_LAG)
        ypool = pool("y", 2)

        proj_ps = pool("proj_ps", int(os.environ.get("PROJ_PS_BUFS", "2")), space="PSUM")
        sim_ps = pool("sim_ps", 1 if UBIG else 2, space="PSUM")
        o_ps = pool("o_ps", int(os.environ.get("O_PS_BUFS", "2")), space="PSUM")

        shared_x = None

        def load_x(yy):
            if "nox" in ABL:
                return shared_x
            ts = []
            for k in range(XK):
                t_ = xpool.tile([128, T], PJ, tag=f"x{k}", name=f"x{k}")
                nc.sync.dma_start(t_[:, :], xw[k * 128:(k + 1) * 128, yy * T:(yy + 1) * T])
                ts.append(t_)
            return ts

        def emit_y(yy, o_sb, groups=(0, 1)):
            if "noy" in ABL:
                return
            # output projection + bias (pipelined one window behind so the
            # PE stream interleaves it with the NEXT window's projections).
            # Two i-tiles pack into one PSUM bank (N=256 halves).
            for it2 in groups:
                # y tiles live in proj_ps (idle at this point of the window),
                # so they don't block the new window's po allocations in o_ps
                py = proj_ps.tile([128, 2 * C], FP, tag="projy", name="py")
                for half in range(2):
                    it = 2 * it2 + half
                    for m in range(4):
                        nc.tensor.matmul(
                            py[:, half * C:(half + 1) * C],
                            lhsT=_r(o_sb[m][:, it * 128:(it + 1) * 128]),
                            rhs=_r(wo_sb[m][:, :]),
                            start=(m == 0),
                            stop=(m == 3),
                        )
                ysb = ypool.tile([128, 2 * C], FP, tag="y", name="ysb")
                if Y_TT_PRIO == "hp":
                    with tc.high_priority():
                        nc.vector.tensor_tensor(
                            ysb[:, :], py[:, :], bias2_bc[:, :], AluOpType.add
                        )
                elif Y_TT_PRIO == "defer":
                    with tc.high_priority(offset=-4000):
                        nc.vector.tensor_tensor(
                            ysb[:, :], py[:, :], bias2_bc[:, :], AluOpType.add
                        )
                else:
                    nc.vector.tensor_tensor(
                        ysb[:, :], py[:, :], bias2_bc[:, :], AluOpType.add
                    )
                if "nostore" not in ABL:
                    dst = out[yy, it2 * 256:(it2 + 1) * 256, :].rearrange(
                        "(h p) c -> p h c", h=2
                    )
                    nc.sync.dma_start(dst, ysb[:, :].rearrange("p (h c) -> p h c", h=2))

        # pin the exp activation-table set while the initial DMAs stream,
        # so the ~2.7us ACT_TABLE_LOAD is off window 0's critical path
        warm = consts.tile([128, 8], FP, tag="warm", name="warm")
        nc.vector.memset(warm[:, :], 0.0)
        nc.scalar.activation(warm[:, :], warm[:, :], Exp, scale=1.0)
        # HAM warmup: dummy matmuls during the initial DMA wait so the PE
        # clock-gate is at 2.4GHz (not cold 1.2) when window 0 starts
        wmm = consts.tile([128, 640], PJ, tag="wmm", name="wmm")
        nc.vector.memset(wmm[:, :].bitcast(FP), 0.0)
        for _ in range(WARM_MMS):
            wps = proj_ps.tile([128, T], FP, tag="projy", name="wps")
            nc.tensor.matmul(
                wps[:, :], lhsT=wmm[:, 0:128], rhs=wmm[:, 128:640],
                start=True, stop=True,
            )

        # ---- weights / constants (loaded once) ----
        wqk_sb = []
        wv_sb = []
        wo_sb = []
        for k in range(XK):
            t_ = consts.tile([128, 2 * INNER], PJ, tag=f"wqk{k}", name=f"wqk{k}")
            nc.sync.dma_start(t_[:, :], wqkT[k * 128:(k + 1) * 128, :])
            wqk_sb.append(t_)
            t_ = consts.tile([128, INNER], PJ, tag=f"wv{k}", name=f"wv{k}")
            nc.sync.dma_start(t_[:, :], wvT[k * 128:(k + 1) * 128, :])
            wv_sb.append(t_)
        for m in range(4):
            t_ = consts.tile([128, C], PJ, tag=f"wo{m}", name=f"wo{m}")
            nc.sync.dma_start(t_[:, :], woT[m * 128:(m + 1) * 128, :])
            wo_sb.append(t_)
        bo_sb = consts.tile([1, C], FP, tag="bo")
        nc.sync.dma_start(bo_sb[:, :], bo[:, :])
        bias_bc = consts.tile([128, C], FP, tag="bias_bc")
        nc.gpsimd.partition_broadcast(bias_bc[:, :], bo_sb[:, :])
        bias2_bc = consts.tile([128, 2 * C], FP, tag="bias2_bc")
        nc.vector.tensor_copy(bias2_bc[:, 0:C], bias_bc[:, :])
        nc.vector.tensor_copy(bias2_bc[:, C:2 * C], bias_bc[:, :])
        ones8 = consts.tile([128, HEADS], BA, tag="ones8")
        nc.scalar.activation(
            ones8[:, :], bias_bc[:, 0:HEADS],
            mybir.ActivationFunctionType.Identity, bias=1.0, scale=0.0,
        )
        ones_rb = consts.tile([64, T], FP, tag="ones_rb")
        nc.vector.memset(ones_rb[:, :], 1.0)
        # prefill the ones column of each 65-wide head slot in every
        # physical v buffer once; the in-loop copies only write the o cols
        ones_col = 0 if V_ONES_FIRST else 64
        for _ in range(3):  # vpool rotates over 3 buffers per tag
            for t in range(4):
                t_ = vpool.tile([128, HEADS * 65], BA, tag=f"v{t}", name=f"vinit{t}")
                nc.vector.tensor_copy(t_[:, ones_col::65], ones8[:, :])

        # Q/K evac copies routed to ACT for these chunk ids (DVE relief)
        qk_act_chunks = QK_ACT_CHUNKS

        def emit_proj_pair(xwy, qk, p):
            # chunks p (Q) and 4+p (K): exactly what head pair p's sims need
            mm_hp = tc.high_priority if (PROJ0_HP and p == 0) else _nullctx
            cp_hp = tc.high_priority if (QK0_HP and p == 0) else _nullctx
            for mm in (p, 4 + p):
                ps = proj_ps.tile([128, T], FP, tag="projy", name="proj")
                for k in range(XK):
                    with mm_hp():
                        nc.tensor.matmul(
                            ps[:, :],
                            lhsT=_r(wqk_sb[k][:, mm * 128:(mm + 1) * 128]),
                            rhs=_r(xwy[k][:, :]),
                            start=(k == 0),
                            stop=(k == XK - 1),
                        )
                t_ = qkpool.tile([128, T], BA, tag=f"qk{mm}", name=f"qk{mm}")
                if SPLIT_QK0 and mm in (0, 4):
                    nc.scalar.copy(t_[:, 0:T // 2], ps[:, 0:T // 2])
                    nc.vector.tensor_copy(t_[:, T // 2:T], ps[:, T // 2:T])
                elif mm in qk_act_chunks:
                    with cp_hp():
                        nc.scalar.copy(t_[:, :], ps[:, :])
                else:
                    with cp_hp():
                        nc.vector.tensor_copy(t_[:, :], ps[:, :])
                qk[mm] = t_

        def emit_sims(qk, m, js=(0, 1, 2, 3), us=None):
            if UBIG:
                return emit_sims_big(qk, m)
            if us is None:
                us = ([None] * 4, [None] * 4)
            for j in js:
                psim = sim_ps.tile([128, 2 * T], FP, tag="sim", name="psim")
                for b in (0, 1):
                    lo, hi = b * 64, (b + 1) * 64
                    nc.tensor.matmul(
                        psim[:, b * T:(b + 1) * T],
                        lhsT=_r(qk[4 + m][lo:hi, j * 128:(j + 1) * 128]),
                        rhs=_r(qk[m][lo:hi, :]),
                        start=True,
                        stop=True,
                    )
                u = upool.tile([128, 2 * T], BA, tag=f"u{j}", name=f"u{j}")
                if "halfexp" in ABL:
                    nc.scalar.activation(u[:, 0:T], psim[:, 0:T], Exp, scale=SCALE)
                else:
                    nc.scalar.activation(u[:, :], psim[:, :], Exp, scale=SCALE)
                us[0][j] = u[:, 0:T]
                us[1][j] = u[:, T:2 * T]
            return us

        def emit_sims_big(qk, m):
            # [128, 4T] psim per j-pair: quarters (b0 j, b1 j, b0 j+1, b1 j+1)
            # land in 4 distinct banks; ONE [128, 2048] exp covers both j's.
            us = ([None] * 4, [None] * 4)
            for jh in range(2):
                psim = sim_ps.tile([128, 4 * T], FP, tag="simbig", name="psimb")
                for dj in range(2):
                    j = 2 * jh + dj
                    for b in (0, 1):
                        lo, hi = b * 64, (b + 1) * 64
                        q = 2 * dj + b
                        nc.tensor.matmul(
                            psim[:, q * T:(q + 1) * T],
                            lhsT=_r(qk[4 + m][lo:hi, j * 128:(j + 1) * 128]),
                            rhs=_r(qk[m][lo:hi, :]),
                            start=True,
                            stop=True,
                        )
                u = upool.tile([128, 4 * T], BA, tag=f"ub{jh}", name=f"ub{jh}")
                nc.scalar.activation(u[:, :], psim[:, :], Exp, scale=SCALE)
                for dj in range(2):
                    j = 2 * jh + dj
                    us[0][j] = u[:, (2 * dj) * T:(2 * dj + 1) * T]
                    us[1][j] = u[:, (2 * dj + 1) * T:(2 * dj + 2) * T]
            return us

        # The 8-window pass is emitted ONCE inside a hardware For_i loop over
        # `repeat`, so the program size (and thus per-dispatch client-side
        # overhead) is constant in `repeat`; the in-NEFF loop makes repeat
        # differencing measure true steady-state device time.
        def emit_pass(passes=1):
            # `passes` > 1 emits one CONTINUOUS multi-pass window stream
            # (window w of pass k+1 pipelines into pass k's tail), so only
            # the For_i back-edge itself drains the pipeline.
            NT = NW * passes
            x_tiles = {0: load_x(0)}
            if NW > 1:
                x_tiles[1] = load_x(1)
            win = {}

            def start_window_proj(idx2):
                xwy2 = x_tiles.pop(idx2)
                qk2 = [None] * 8
                emit_proj_pair(xwy2, qk2, 0)
                win[idx2] = {"xwy": xwy2, "qk": qk2, "uss": {}}

            def start_window_sims(idx2):
                st2 = win[idx2]
                st2["uss"][0] = emit_sims(st2["qk"], 0)

            def start_window(idx2):
                start_window_proj(idx2)
                start_window_sims(idx2)

            start_window(0)
            prevs = []  # (y, o_sb) of windows whose y-projection is pending
            for idx in range(NT):
                y = idx % NW
                # prefetch the x slab two windows ahead (keeps loads ahead of
                # stores in the sync DMA queue)
                if idx + 2 < NT:
                    x_tiles[idx + 2] = load_x((idx + 2) % NW)
                st = win.pop(idx)
                xwy, qk, uss = st["xwy"], st["qk"], st["uss"]

                # remaining Q/K projection pairs (pair 0 was emitted by
                # start_window during the previous window, closing the ACT
                # bubble at the window boundary)
                for p in (1, 2, 3):
                    emit_proj_pair(xwy, qk, p)

                # ---- V projection, token-major with ones column per head ----
                # v[t][token, h*65 + d], col h*65+64 == 1.0
                v = []
                for t in range(4):
                    ps = proj_ps.tile([128, INNER], FP, tag="projy", name="projv")
                    for k in range(XK):
                        nc.tensor.matmul(
                            ps[:, :],
                            lhsT=_r(xwy[k][:, t * 128:(t + 1) * 128]),
                            rhs=_r(wv_sb[k][:, :]),
                            start=(k == 0),
                            stop=(k == XK - 1),
                        )
                    t_ = vpool.tile([128, HEADS * 65], BA, tag=f"v{t}", name=f"v{t}")
                    dst = t_[:, :].rearrange("p (h e) -> p h e", e=65)
                    src = ps[:, :].rearrange("p (h e) -> p h e", e=64)
                    if V_ACT and t in V_ACT_TS:
                        nc.scalar.copy(dst[:, :, ORO:ORO + 64], src)
                    else:
                        nc.vector.tensor_copy(dst[:, :, ORO:ORO + 64], src)
                    v.append(t_)

                # ---- attention, head pairs (2m, 2m+1) with row-packed sim ----
                # The b=0 / b=1 sim matmuls use complementary 64-partition row
                # groups (auto tile_position from base_partition), so the PE can
                # run them concurrently.
                o_sb = [opool.tile([128, T], PJ, tag=f"o{m}", name=f"o{m}") for m in range(4)]

                # sim+exp stage software-pipelined one pair ahead of the O stage;
                # at the last pair the NEXT window's preamble (proj pair 0 +
                # pair-0 sims) is emitted instead, so ACT's exp stream never
                # drains -- not even across the window boundary
                for m in range(4):
                    nxt = None
                    if m < 3:
                        if SIMS_SPLIT:
                            nxt = emit_sims(qk, m + 1, js=(0, 1))
                        else:
                            uss[m + 1] = emit_sims(qk, m + 1)
                    else:
                        if idx + 1 < NT:
                            if START_EARLY:
                                start_window_sims(idx + 1)
                            else:
                                start_window(idx + 1)
                    if START_EARLY and m == 2 and idx + 1 < NT:
                        start_window_proj(idx + 1)
                    us = uss.pop(m)
                    # previous window's output projection: one PSUM group per
                    # Y_MS slot, so its 16 matmuls don't burst into
                    # the PE queue all at once ahead of the sims
                    if m in Y_MS and len(prevs) >= Y_LAG:
                        emit_y(prevs[0][0], prevs[0][1], groups=(Y_MS.index(m),))
                        if m == Y_MS[-1]:
                            prevs.pop(0)
                    s2 = pos = None
                    if S_PAIR and "nonorm" not in ABL and "nobcast" not in ABL:
                        s2 = rpool.tile([2, T], FP, tag="s2", name="s2")
                        pos = []
                    for b in (0, 1):
                        h = 2 * m + b
                        lo, hi = b * 64, (b + 1) * 64
                        po = o_ps.tile([65, T], FP, tag="o_ps", name="po")
                        for j in range(4):
                            nc.tensor.matmul(
                                po[:, :],
                                lhsT=_r(v[j][:, h * 65:(h + 1) * 65]),
                                rhs=_r(us[b][j]),
                                start=(j == 0),
                                stop=(j == 3),
                            )
                        if s2 is not None:
                            # stage this head's denominators into row b of the
                            # pair tile; the pair chain runs after b==1
                            with tc.high_priority():
                                nc.vector.tensor_copy(
                                    s2[b:b + 1, :], po[SROW:SROW + 1, :]
                                )
                            pos.append(po)
                            if b == 0 and nxt is not None:
                                emit_sims(qk, m + 1, js=(2, 3), us=nxt)
                                uss[m + 1] = nxt
                                nxt = None
                            if b == 1:
                                r2 = rpool.tile([2, T], FP, tag="r2", name="r2")
                                with tc.high_priority():
                                    nc.vector.reciprocal_approx_fast(
                                        r2[:, :], s2[:, :]
                                    )
                                for b2 in (0, 1):
                                    lo2, hi2 = b2 * 64, (b2 + 1) * 64
                                    rb = rpool.tile([64, T], FP, tag="rb", name="rb")
                                    if HP_BCAST:
                                        with tc.high_priority():
                                            nc.gpsimd.partition_broadcast(
                                                rb[:, :], r2[b2:b2 + 1, :]
                                            )
                                    else:
                                        nc.gpsimd.partition_broadcast(
                                            rb[:, :], r2[b2:b2 + 1, :]
                                        )
                                    with tc.high_priority():
                                        nc.vector.tensor_tensor(
                                            o_sb[m][lo2:hi2, :],
                                            pos[b2][ORO:ORO + 64, :],
                                            rb[:, :],
                                            AluOpType.mult,
                                        )
                            continue
                        # Two high-priority copies evacuate po (o rows ->
                        # pc_o at base partition 0; s row -> s1 row 0), freeing
                        # the PSUM slot in ~1us so the next pair's O matmuls
                        # never stall. The normalize (approx-recip -> GPSIMD
                        # broadcast -> mult) then runs SBUF-side at base
                        # partition 0 everywhere (custom-op offset bugs and
                        # the NCC equal-base-partition rule both demand it),
                        # off the critical path (o_sb isn't read until the
                        # next window's emit_y).
                        chp = tc.high_priority if CHAIN_HP else _nullctx
                        s1 = None
                        if not RECIP_PSUM:
                            s1 = rpool.tile([1, T], FP, tag="s1", name="s1")
                            with chp():
                                nc.vector.tensor_copy(s1[:, :], po[SROW:SROW + 1, :])
                        if "nonorm" in ABL:
                            nc.vector.tensor_copy(o_sb[m][lo:hi, :], po[ORO:ORO + 64, :])
                        elif "nobcast" in ABL:
                            nc.vector.tensor_tensor(
                                o_sb[m][lo:hi, :], po[ORO:ORO + 64, :], ones_rb[:, :],
                                AluOpType.mult,
                            )
                        else:
                            # reciprocal_approx_fast: ~51 ULP, ~5x faster than
                            # the 3.5us full-precision DVE reciprocal on HW.
                            # The mult reads po straight from PSUM (mixed
                            # PSUM+SBUF operands are legal) and is what frees
                            # the po slot; s goes to SBUF first only because
                            # the approx-recip custom op misreads partition
                            # offsets (so s must sit at a base-0 SBUF row).
                            r1 = rpool.tile([1, T], FP, tag="r1", name="r1")
                            with chp():
                                nc.vector.reciprocal_approx_fast(
                                    r1[:, :],
                                    po[SROW:SROW + 1, :] if RECIP_PSUM else s1[:, :],
                                )
                            rb = rpool.tile([64, T], FP, tag="rb", name="rb")
                            if HP_BCAST:
                                with tc.high_priority():
                                    nc.gpsimd.partition_broadcast(rb[:, :], r1[:, :])
                            else:
                                nc.gpsimd.partition_broadcast(rb[:, :], r1[:, :])
                            with chp():
                                nc.vector.tensor_tensor(
                                    o_sb[m][lo:hi, :], po[ORO:ORO + 64, :], rb[:, :],
                                    AluOpType.mult,
                                )
                        if b == 0 and nxt is not None:
                            # second half of the next pair's sims lands between
                            # this pair's two O groups
                            emit_sims(qk, m + 1, js=(2, 3), us=nxt)
                            uss[m + 1] = nxt
                            nxt = None

                prevs.append((y, o_sb))

            for pv in prevs:
                emit_y(*pv)

        if "nox" in ABL:
            sx = []
            for k in range(XK):
                t_ = consts.tile([128, T], PJ, tag=f"sx{k}", name=f"sx{k}")
                nc.sync.dma_start(t_[:, :], xw[k * 128:(k + 1) * 128, 0:T])
                sx.append(t_)
            shared_x = sx

        if use_loop and repeat > body_passes:
            trips, rem = divmod(repeat, body_passes)
            with tc.For_i(0, trips, hint_engines=mybir.ALL_ENGINES,
                          staggered_reset=True):
                emit_pass(body_passes)
            if rem:
                emit_pass(rem)
        else:
            # repeat <= body_passes (notably the harness's repeat=1 path):
            # straight-line emission -- no For_i entry/stage/back-edge
            # machinery on the single-dispatch span
            emit_pass(repeat)

    nc.compile()
    return nc


def _get_nc():
    key = ("nc", BF16_ATT)
    if key not in _CACHE:
        _CACHE[key] = _build()
    return _CACHE[key]


def _host_prep(x, wq, wkv, wo, bo):
    x = np.asarray(x, dtype=np.float32)
    wq = np.asarray(wq, dtype=np.float32)
    wkv = np.asarray(wkv, dtype=np.float32)
    wo = np.asarray(wo, dtype=np.float32)
    bo = np.asarray(bo, dtype=np.float32)

    pj = mybir.dt.np(mybir.dt.bfloat16) if PROJ_BF16 else np.float32

    wk = wkv[:INNER]
    wv = wkv[INNER:]
    wqkT = np.ascontiguousarray(np.concatenate([wq, wk], axis=0).T).astype(pj)
    wvT = np.ascontiguousarray(wv.T).astype(pj)  # (256, 512)
    woT = np.ascontiguousarray(wo.T).astype(pj)  # (512, 256)
    bo2 = np.ascontiguousarray(bo.reshape(1, C))

    x0 = x[0]  # (256, 2, 128, 128)
    in_maps = []
    for c in range(N_CORES):
        xc = x0[:, :, c * WS:(c + 1) * WS, :]  # (256, 2, 16, 128)
        xc = xc.reshape(C, F, WS, NW, WS).transpose(0, 3, 1, 2, 4)  # (C, y, f, r, wl)
        xc = np.ascontiguousarray(xc.reshape(C, NPIX)).astype(pj)
        in_maps.append({"xw": xc, "wqkT": wqkT, "wvT": wvT, "woT": woT, "bo": bo2})
    return in_maps


def _assemble(results):
    # per-core "out" is (NW, T, C) = (y, (f, r, wl), co); core c covers H rows
    # [16c, 16c+16).
    full = np.empty((1, C, F, H, W), dtype=np.float32)
    for c in range(N_CORES):
        oc = results[c]["out"]  # (8, 512, 256)
        oc = oc.reshape(NW, F, WS, WS, C).transpose(4, 1, 2, 0, 3)  # (C,f,r,y,wl)
        full[0, :, :, c * WS:(c + 1) * WS, :] = oc.reshape(C, F, WS, W)
    return full


def run(inputs, trace=False):
    nc = _get_nc()
    in_maps = _host_prep(**inputs)
    res = run_bass_kernel_spmd(
        nc, in_maps, core_ids=list(range(N_CORES)), trace=trace
    )
    out = _assemble(res.results)
    return out, res.exec_time_ns


def bench(inputs, iters=3):
    """Correct output + min wall-clock of the device execution (ns).

    No NTFF profiling hook exists in this environment, so the best available
    hardware number is wall time of the PJRT dispatch (includes axon tunnel
    overhead; min over iters approximates steady-state)."""
    import time

    nc = _get_nc()
    in_maps = _host_prep(**inputs)
    out = None
    best = None
    for _ in range(iters):
        t0 = time.perf_counter()
        res = run_bass_kernel_spmd(nc, in_maps, core_ids=list(range(N_CORES)))
        dt = (time.perf_counter() - t0) * 1e9
        best = dt if best is None else min(best, dt)
        out = _assemble(res.results)
    return out, best


def kernel(**inputs):
    out, _ = run(inputs, trace=False)
    return out


if __name__ == "__main__":
    rng = np.random.default_rng(0)
    ins = {
        "x": rng.standard_normal((1, C, F, H, W), dtype=np.float32),
        "wq": rng.standard_normal((INNER, C), dtype=np.float32) * C ** -0.5,
        "wkv": rng.standard_normal((2 * INNER, C), dtype=np.float32) * C ** -0.5,
        "wo": rng.standard_normal((C, INNER), dtype=np.float32) * INNER ** -0.5,
        "bo": np.zeros((C,), dtype=np.float32),
    }
    out = kernel(**ins)
    print(out.shape, out.dtype)

